# revision 11
# baseline (speedup 1.0000x reference)
"""Trainium2 Bass kernel for nn_AdapterBlock (cross-attention adapter block).

Reference computation (per batch b):
    x   = concat(h[b], vis[b])                    # [S=3072, C=768]
    q   = h @ Wq.T ; k = x @ Wk.T ; v = x @ Wv.T  # H=8 heads, D=96
    att = softmax(q k^T / sqrt(D)) v              # [L, C]
    out = LayerNorm(att @ Wo.T) * gamma + beta

Sharding: 8 cores = (batch b = core//2) x (query-half = core%2). Each core
computes K/V for the full x[b] (duplicated across the pair) and attention +
output projection + LayerNorm for its own 1024 query rows. No collectives.

Device algorithm (per core), fp32 storage with float32r matmuls (1 cyc/row at
N>=256) and bf16 for the probability@V matmul:
  Phase A: PE-transpose x chunks -> xT; project kT (d-major, spilled to a DRAM
           scratch), V (SBUF-resident bf16, per-head 97-wide blocks with a
           ones column appended -> PV matmul also yields the softmax
           denominator), qT (SBUF-resident per-head [96, 1024]).
  Phase B: per (head, s-tile): scoresT[s,l] = kT_h^T-style matmul; exp on the
           scalar engine (scale folded in, no max-subtraction -- scores are
           O(1) by construction); PV accumulates out_pvT[d+1, l] in PSUM over
           all 24 s-tiles.  Normalize by the ones-column denominator.
  Phase C: out[l, c] = sum_h attn_outT_h^T @ WoT_h rows, LayerNorm fused on
           the PSUM result, DMA out.
"""

import os
from contextlib import ExitStack

import numpy as np

import concourse.bass as bass
import concourse.bacc as bacc
import concourse.mybir as mybir
import concourse.tile as tile
from concourse.bass_utils import run_bass_kernel_spmd
from concourse.masks import make_identity

F32 = mybir.dt.float32
F32R = mybir.dt.float32r
BF16 = mybir.dt.bfloat16
AF = mybir.ActivationFunctionType

B, L, NV, C = 4, 2048, 1024, 768
H, D = 8, 96
S = L + NV            # 3072 kv tokens
LLOC = L // 2         # 1024 query rows per core
SCALE = float(D) ** -0.5
N_CORES = 8
CHUNK = 256           # phase-A token chunk
NST = S // 128        # 24 s-tiles
EPS = 1e-5


def _r(ap):
    """identity (bf16 pipeline; fp32r path hit a walrus sync-wait limit)."""
    return ap


def build_nc():
    nc = bacc.Bacc("TRN2", target_bir_lowering=False, num_devices=N_CORES)
    x_in = nc.declare_dram_parameter("x", [S, C], BF16, isOutput=False)
    hq_in = nc.declare_dram_parameter("hq", [LLOC, C], BF16, isOutput=False)
    wqt_in = nc.declare_dram_parameter("WqT", [C, C], BF16, isOutput=False)
    wkt_in = nc.declare_dram_parameter("WkT", [C, C], BF16, isOutput=False)
    wvt_in = nc.declare_dram_parameter("WvT", [C, C], BF16, isOutput=False)
    wot_in = nc.declare_dram_parameter("WoT", [C, C], BF16, isOutput=False)
    gmb_in = nc.declare_dram_parameter("gammab", [128, C], F32, isOutput=False)
    btb_in = nc.declare_dram_parameter("betab", [128, C], F32, isOutput=False)
    out_ext = nc.declare_dram_parameter("out", [LLOC, C], F32, isOutput=True)
    # d-major kT spill: [s-tile, d, s-within-tile]
    kt_dram = nc.dram_tensor("kt_scratch", [NST, C, 128], BF16)

    with tile.TileContext(nc) as tc, ExitStack() as ctx:
        const_pool = ctx.enter_context(tc.tile_pool(name="const", bufs=1))
        ident = const_pool.tile([128, 128], BF16, tag="ident")
        make_identity(nc, ident[:])
        gamma_t = const_pool.tile([128, C], F32, tag="gamma")
        beta_t = const_pool.tile([128, C], F32, tag="beta")
        nc.sync.dma_start(gamma_t[:], gmb_in[:])
        nc.sync.dma_start(beta_t[:], btb_in[:])

        # persistent tensors (live through phase C)
        qt_pool = ctx.enter_context(tc.tile_pool(name="qt", bufs=1))
        qt_tiles = [qt_pool.tile([D, LLOC], BF16, tag=f"qt{h}", name=f"qt{h}")
                    for h in range(H)]
        v_pool = ctx.enter_context(tc.tile_pool(name="v", bufs=1))
        v_tiles = [v_pool.tile([128, H * (D + 1)], BF16, tag=f"v{t}", name=f"v{t}")
                   for t in range(NST)]
        ao_pool = ctx.enter_context(tc.tile_pool(name="ao", bufs=1))
        ao_tiles = [ao_pool.tile([D, LLOC], BF16, tag=f"ao{h}", name=f"ao{h}")
                    for h in range(H)]

        # ones columns of the augmented V (denominator trick)
        for t in range(NST):
            nc.gpsimd.memset(
                v_tiles[t][:].rearrange("p (g c) -> p g c", c=D + 1)[:, :, D], 1.0)

        # ---------------- Phase A: transpose + projections ----------------
        with tc.tile_pool(name="wproj", bufs=1) as wp, \
             tc.tile_pool(name="xa", bufs=2) as xa_pool, \
             tc.tile_pool(name="xt", bufs=2) as xt_pool, \
             tc.tile_pool(name="stg", bufs=3) as stg_pool, \
             tc.tile_pool(name="tp_ps", bufs=2, space="PSUM") as tp_ps, \
             tc.tile_pool(name="kt_ps", bufs=2, space="PSUM") as kt_ps, \
             tc.tile_pool(name="v_ps", bufs=2, space="PSUM") as v_ps, \
             tc.tile_pool(name="qt_ps", bufs=2, space="PSUM") as qt_ps:
            wq_t, wk_t, wv_t = [], [], []
            for ci in range(6):
                for lst, srcw, nm in ((wq_t, wqt_in, "q"), (wk_t, wkt_in, "k"),
                                      (wv_t, wvt_in, "v")):
                    t = wp.tile([128, C], BF16, tag=f"w{nm}{ci}", name=f"w{nm}{ci}")
                    nc.sync.dma_start(t[:], srcw[128 * ci:128 * (ci + 1), :])
                    lst.append(t)
            for ch in range(16):
                is_q = ch >= 12
                src = hq_in if is_q else x_in
                base = (ch - 12) * CHUNK if is_q else ch * CHUNK
                xa = []
                for j in range(2):
                    t = xa_pool.tile([128, C], BF16, tag=f"xa{j}", name=f"xa{j}")
                    nc.sync.dma_start(
                        t[:], src[base + 128 * j:base + 128 * (j + 1), :])
                    xa.append(t)
                xts = []
                for ci in range(6):
                    tp = tp_ps.tile([128, CHUNK], BF16, tag="tp")
                    for j in range(2):
                        nc.tensor.transpose(
                            tp[:, 128 * j:128 * (j + 1)],
                            xa[j][:, 128 * ci:128 * (ci + 1)], ident[:])
                    xt = xt_pool.tile([128, CHUNK], BF16, tag=f"xt{ci}",
                                      name=f"xt{ci}")
                    nc.vector.tensor_copy(xt[:], tp[:])
                    xts.append(xt)
                if not is_q:
                    # kT (d-major) -> DRAM spill
                    for di in range(6):
                        ktp = kt_ps.tile([128, CHUNK], F32, tag="kt")
                        for ci in range(6):
                            nc.tensor.matmul(
                                ktp[:],
                                _r(wk_t[ci][:, 128 * di:128 * (di + 1)]),
                                _r(xts[ci][:]),
                                start=(ci == 0), stop=(ci == 5))
                        stg = stg_pool.tile([128, CHUNK], BF16, tag="ktstg")
                        nc.scalar.copy(stg[:], ktp[:])
                        for j in range(2):
                            nc.sync.dma_start(
                                kt_dram[2 * ch + j, 128 * di:128 * (di + 1), :],
                                stg[:, 128 * j:128 * (j + 1)])
                    # V (natural, bf16, heads interleaved at stride 97)
                    for j in range(2):
                        for hf in range(2):
                            vp = v_ps.tile([128, 384], F32, tag="vps")
                            for ci in range(6):
                                nc.tensor.matmul(
                                    vp[:],
                                    _r(xts[ci][:, 128 * j:128 * (j + 1)]),
                                    _r(wv_t[ci][:, 384 * hf:384 * (hf + 1)]),
                                    start=(ci == 0), stop=(ci == 5))
                            dst = v_tiles[2 * ch + j][:].rearrange(
                                "p (g c) -> p g c", c=D + 1)[:, 4 * hf:4 * hf + 4, 0:D]
                            nc.scalar.copy(
                                dst, vp[:].rearrange("p (g c) -> p g c", c=D))
                else:
                    lbase = (ch - 12) * CHUNK
                    for h in range(H):
                        qp = qt_ps.tile([D, CHUNK], F32, tag="qt")
                        for ci in range(6):
                            nc.tensor.matmul(
                                qp[:],
                                _r(wq_t[ci][:, D * h:D * (h + 1)]),
                                _r(xts[ci][:]),
                                start=(ci == 0), stop=(ci == 5))
                        nc.scalar.copy(
                            qt_tiles[h][:, lbase:lbase + CHUNK], qp[:])

        # WoT per-head tiles (loaded after phase A frees SBUF; used in C)
        wo_pool = ctx.enter_context(tc.tile_pool(name="wo", bufs=1))
        wot_tiles = []
        for h in range(H):
            t = wo_pool.tile([D, C], BF16, tag=f"wo{h}", name=f"wo{h}")
            nc.sync.dma_start(t[:], wot_in[D * h:D * (h + 1), :])
            wot_tiles.append(t)

        # ---------------- Phase B: attention ----------------
        with tc.tile_pool(name="ktb", bufs=3) as kt_pool, \
             tc.tile_pool(name="ex", bufs=3) as ex_pool, \
             tc.tile_pool(name="nrm", bufs=2) as nrm_pool, \
             tc.tile_pool(name="qk_ps", bufs=2, space="PSUM") as qk_ps, \
             tc.tile_pool(name="pv_ps", bufs=4, space="PSUM") as pv_ps:
            for h in range(H):
                pv = [pv_ps.tile([D + 1, 512], F32, tag="pv", name="pv")
                      for _ in range(2)]
                for t in range(NST):
                    kt = kt_pool.tile([D, 128], BF16, tag="kt")
                    nc.sync.dma_start(kt[:], kt_dram[t, D * h:D * (h + 1), :])
                    qk = qk_ps.tile([128, 1024], F32, tag="qk")
                    for l in range(2):
                        nc.tensor.matmul(
                            qk[:, 512 * l:512 * (l + 1)], _r(kt[:]),
                            _r(qt_tiles[h][:, 512 * l:512 * (l + 1)]),
                            start=True, stop=True)
                    ex = ex_pool.tile([128, 1024], BF16, tag="ex")
                    nc.scalar.activation(ex[:], qk[:], AF.Exp, scale=SCALE)
                    vh = v_tiles[t][:].rearrange(
                        "p (g c) -> p g c", c=D + 1)[:, h, :]
                    for l in range(2):
                        nc.tensor.matmul(
                            pv[l][:], vh, ex[:, 512 * l:512 * (l + 1)],
                            start=(t == 0), stop=(t == NST - 1))
                # normalize by denominator (row D of pv) and store attn_outT
                for l in range(2):
                    rec = nrm_pool.tile([1, 512], F32, tag="rec")
                    nc.vector.reciprocal(rec[:], pv[l][D:D + 1, :])
                    bc = nrm_pool.tile([D, 512], F32, tag="bc")
                    nc.gpsimd.partition_broadcast(bc[:], rec[:])
                    nc.vector.tensor_mul(
                        ao_tiles[h][:, 512 * l:512 * (l + 1)],
                        pv[l][0:D, :], bc[:])

        # ---------------- Phase C: out-proj + LayerNorm ----------------
        with tc.tile_pool(name="ln", bufs=2) as ln_pool, \
             tc.tile_pool(name="wo_ps", bufs=2, space="PSUM") as wo_ps:
            for lt in range(LLOC // 128):
                wp2 = wo_ps.tile([128, C], F32, tag="wop")
                for n0, n1 in ((0, 512), (512, C)):
                    for h in range(H):
                        nc.tensor.matmul(
                            wp2[:, n0:n1],
                            _r(ao_tiles[h][:, 128 * lt:128 * (lt + 1)]),
                            _r(wot_tiles[h][:, n0:n1]),
                            start=(h == 0), stop=(h == H - 1))
                st6 = ln_pool.tile([128, 2, 6], F32, tag="st6")
                nc.vector.bn_stats(st6[:, 0, :], wp2[:, 0:384])
                nc.vector.bn_stats(st6[:, 1, :], wp2[:, 384:C])
                st2 = ln_pool.tile([128, 2], F32, tag="st2")
                nc.vector.bn_aggr(st2[:], st6[:])
                veps = ln_pool.tile([128, 1], F32, tag="veps")
                nc.vector.tensor_scalar_add(veps[:], st2[:, 1:2], EPS)
                std = ln_pool.tile([128, 1], F32, tag="std")
                nc.scalar.activation(std[:], veps[:], AF.Sqrt)
                rstd = ln_pool.tile([128, 1], F32, tag="rstd")
                nc.vector.reciprocal(rstd[:], std[:])
                negmu = ln_pool.tile([128, 1], F32, tag="negmu")
                nc.vector.tensor_scalar_mul(negmu[:], st2[:, 0:1], -1.0)
                t1 = ln_pool.tile([128, C], F32, tag="t1")
                nc.vector.scalar_tensor_tensor(
                    t1[:], wp2[:], negmu[:], gamma_t[:],
                    op0=mybir.AluOpType.add, op1=mybir.AluOpType.mult)
                ot = ln_pool.tile([128, C], F32, tag="ot")
                nc.vector.scalar_tensor_tensor(
                    ot[:], t1[:], rstd[:], beta_t[:],
                    op0=mybir.AluOpType.mult, op1=mybir.AluOpType.add)
                nc.sync.dma_start(out_ext[128 * lt:128 * (lt + 1), :], ot[:])
    nc.finalize()
    return nc


_CACHE = {}


def _get_nc():
    if "nc" not in _CACHE:
        _CACHE["nc"] = build_nc()
    return _CACHE["nc"]


def make_in_maps(h, vis, Wq, Wk, Wv, Wo, ln_gamma, ln_beta):
    import ml_dtypes
    bf16 = ml_dtypes.bfloat16
    h = np.asarray(h, np.float32).astype(bf16)
    vis = np.asarray(vis, np.float32).astype(bf16)
    wqt = np.ascontiguousarray(np.asarray(Wq, np.float32).T.astype(bf16))
    wkt = np.ascontiguousarray(np.asarray(Wk, np.float32).T.astype(bf16))
    wvt = np.ascontiguousarray(np.asarray(Wv, np.float32).T.astype(bf16))
    wot = np.ascontiguousarray(np.asarray(Wo, np.float32).T.astype(bf16))
    gmb = np.ascontiguousarray(
        np.tile(np.asarray(ln_gamma, np.float32)[None, :], (128, 1)))
    btb = np.ascontiguousarray(
        np.tile(np.asarray(ln_beta, np.float32)[None, :], (128, 1)))
    in_maps = []
    for core in range(N_CORES):
        b, half = core // 2, core % 2
        in_maps.append({
            "x": np.ascontiguousarray(np.concatenate([h[b], vis[b]], axis=0)),
            "hq": np.ascontiguousarray(h[b, half * LLOC:(half + 1) * LLOC]),
            "WqT": wqt, "WkT": wkt, "WvT": wvt, "WoT": wot,
            "gammab": gmb, "betab": btb,
        })
    return in_maps


def run(in_maps, trace=False, **kw):
    nc = _get_nc()
    return run_bass_kernel_spmd(nc, in_maps, core_ids=list(range(N_CORES)),
                                trace=trace, **kw)


def assemble(results):
    full = np.empty((B, L, C), np.float32)
    for core in range(N_CORES):
        b, half = core // 2, core % 2
        full[b, half * LLOC:(half + 1) * LLOC] = results[core]["out"]
    return full


def kernel(h, vis, Wq, Wk, Wv, Wo, ln_gamma, ln_beta):
    in_maps = make_in_maps(h, vis, Wq, Wk, Wv, Wo, ln_gamma, ln_beta)
    res = run(in_maps, trace=False)
    return assemble(res.results)


# revision 39
# speedup vs baseline: 17.6431x; 17.6431x over previous
"""Trainium2 Bass kernel for nn_AdapterBlock (cross-attention adapter block).

Reference computation (per batch b):
    x   = concat(h[b], vis[b])                    # [S=3072, C=768]
    q   = h @ Wq.T ; k = x @ Wk.T ; v = x @ Wv.T  # H=8 heads, D=96
    att = softmax(q k^T / sqrt(D)) v              # [L, C]
    out = LayerNorm(att @ Wo.T) * gamma + beta

Sharding: 8 cores = (batch b = core//2) x (query-half = core%2). Each core
computes K/V for the full x[b] (duplicated across the pair) and attention +
output projection + LayerNorm for its own 1024 query rows. No collectives.

All matmul operands are bf16 (inputs/weights quantized host-side; PSUM
accumulation stays fp32), which keeps the TensorEngine at 1 cycle/row and the
final relative error ~5e-3.  Layout is feature-major ("transposed") end to
end so every matmul contracts over the partition axis without runtime
transposes of activations beyond the initial PE-transpose of x:

  A0: PE-transpose hq chunks; project qT_h [96, 1024] per head.
  A1: per 512-token x chunk: PE-transpose into resident xT [768, 3072];
      project V (bf16, per-head 97-wide blocks whose extra ones-column makes
      the PV matmul emit the softmax denominator for free); project head 0's
      kT riding along to keep the PE dense.
  B:  per head: scoresT[s,128 x l,1024] = kT_h slice (stationary) x qT_h;
      exp on ScalarE with the 1/sqrt(D) scale folded in (no max-subtraction:
      scores are O(1) by construction); PV accumulates out_pvT[97, 512] over
      all 24 s-tiles in PSUM; the next head's kT projection is spread evenly
      across the 24 iterations so PE and ScalarE stay balanced.  Normalize
      with the ones-column denominator (reciprocal + partition-broadcast).
  C:  out[l,c] = sum_h ao_h^T @ WoT_h rows (natural orientation, so LayerNorm
      needs no final transpose), LayerNorm fused on the PSUM result, DMA out.

Cost-model timeline: ~370 us/core, TensorEngine ~87% busy (324 us).
"""

from contextlib import ExitStack

import numpy as np

import concourse.bacc as bacc
import concourse.mybir as mybir
import concourse.tile as tile
from concourse.bass_utils import run_bass_kernel_spmd

F32 = mybir.dt.float32
BF16 = mybir.dt.bfloat16
AF = mybir.ActivationFunctionType

B, L, NV, C = 4, 2048, 1024, 768
H, D = 8, 96
S = L + NV            # 3072 kv tokens
LLOC = L // 2         # 1024 query rows per core
SCALE = float(D) ** -0.5
N_CORES = 8
CHUNK = 512           # phase-A token chunk
NST = S // 128        # 24 s-tiles
EPS = 1e-5


def build_nc():
    nc = bacc.Bacc("TRN2", target_bir_lowering=False, num_devices=N_CORES)
    x_in = nc.declare_dram_parameter("x", [S, C], BF16, isOutput=False)
    hq_in = nc.declare_dram_parameter("hq", [LLOC, C], BF16, isOutput=False)
    wqt_in = nc.declare_dram_parameter("WqT", [C, C], BF16, isOutput=False)
    wkt_in = nc.declare_dram_parameter("WkT", [C, C], BF16, isOutput=False)
    wvt_in = nc.declare_dram_parameter("WvT", [C, C], BF16, isOutput=False)
    wot_in = nc.declare_dram_parameter("WoT", [C, C], BF16, isOutput=False)
    gmb_in = nc.declare_dram_parameter("gammab", [128, C], F32, isOutput=False)
    id_in = nc.declare_dram_parameter("ident", [128, 128], BF16, isOutput=False)
    btb_in = nc.declare_dram_parameter("betab", [128, C], F32, isOutput=False)
    out_ext = nc.declare_dram_parameter("out", [LLOC, C], F32, isOutput=True)

    NSC = S // CHUNK      # 6 x-chunks
    NQC = LLOC // CHUNK   # 2 hq-chunks

    with tile.TileContext(nc) as tc, ExitStack() as ctx:
        const_pool = ctx.enter_context(tc.tile_pool(name="const", bufs=1))
        ident = const_pool.tile([128, 128], BF16, tag="ident")
        nc.gpsimd.dma_start(ident[:], id_in[:])
        gamma_t = const_pool.tile([128, C], F32, tag="gamma")
        beta_t = const_pool.tile([128, C], F32, tag="beta")

        # persistent tensors
        qt_pool = ctx.enter_context(tc.tile_pool(name="qt", bufs=1))
        qt_tiles = [qt_pool.tile([D, LLOC], BF16, tag=f"qt{h}", name=f"qt{h}")
                    for h in range(H)]
        v_pool = ctx.enter_context(tc.tile_pool(name="v", bufs=1))
        v_tiles = [v_pool.tile([128, H * (D + 1)], BF16, tag=f"v{t}", name=f"v{t}")
                   for t in range(NST)]
        kt_pool = ctx.enter_context(tc.tile_pool(name="ktp", bufs=1))
        kt_tiles = [kt_pool.tile([D, S], BF16, tag=f"kt{h}", name=f"kt{h}")
                    for h in range(H)]
        ao_pool = ctx.enter_context(tc.tile_pool(name="ao", bufs=1))
        ao_tiles = [ao_pool.tile([D, LLOC], BF16, tag=f"ao{h}", name=f"ao{h}")
                    for h in range(H)]

        # xT (transposed x) and Wk stay live through the per-head loop
        with tc.tile_pool(name="xT", bufs=1) as xT_pool, \
             tc.tile_pool(name="wk", bufs=1) as wk_pool:
            xT = [xT_pool.tile([128, S], BF16, tag=f"xT{ci}", name=f"xT{ci}")
                  for ci in range(6)]
            kt_ps_outer = ctx.enter_context(
                tc.tile_pool(name="kt_ps", bufs=2, space="PSUM"))

            def project_kt_steps(h, sc, cis, state):
                # run a subset of kT(h) chunk sc's accumulation steps
                if 0 in cis:
                    state[sc] = kt_ps_outer.tile([D, CHUNK], F32, tag="kt",
                                                 name="ktp")
                ktp = state[sc]
                for ci in cis:
                    nc.tensor.matmul(
                        ktp[:], wk_t[ci][:, D * h:D * (h + 1)],
                        xT[ci][:, sc * CHUNK:(sc + 1) * CHUNK],
                        start=(ci == 0), stop=(ci == 5))
                if 5 in cis:
                    nc.vector.tensor_copy(
                        kt_tiles[h][:, sc * CHUNK:(sc + 1) * CHUNK], ktp[:])
            wk_t = [wk_pool.tile([128, C], BF16, tag=f"wk{ci}", name=f"wk{ci}")
                    for ci in range(6)]

            # ---- A0: transpose hq, project qT (wq scope closes before wv opens) ----
            with tc.tile_pool(name="wq", bufs=1) as wq_pool, \
                 tc.tile_pool(name="xaq", bufs=2) as xaq_pool, \
                 tc.tile_pool(name="xtq", bufs=2) as xtq_pool, \
                 tc.tile_pool(name="tp_ps", bufs=3, space="PSUM") as tp_ps, \
                 tc.tile_pool(name="qt_ps", bufs=2, space="PSUM") as qt_ps:
                xa_pre = []
                for j in range(4):
                    t = xaq_pool.tile([128, C], BF16, tag=f"xa{j}", name=f"xa{j}")
                    nc.sync.dma_start(t[:], hq_in[128 * j:128 * (j + 1), :])
                    xa_pre.append(t)
                wq_t = []
                for ci in range(6):
                    t = wq_pool.tile([128, C], BF16, tag=f"wq{ci}", name=f"wq{ci}")
                    nc.sync.dma_start(t[:], wqt_in[128 * ci:128 * (ci + 1), :])
                    wq_t.append(t)
                for ch in range(NQC):
                    if ch == 0:
                        xa = xa_pre
                    else:
                        xa = []
                        for j in range(4):
                            t = xaq_pool.tile([128, C], BF16, tag=f"xa{j}",
                                              name=f"xa{j}")
                            nc.sync.dma_start(
                                t[:],
                                hq_in[ch * CHUNK + 128 * j:ch * CHUNK + 128 * (j + 1), :])
                            xa.append(t)
                    xts = []
                    for ci in range(6):
                        tp = tp_ps.tile([128, CHUNK], BF16, tag="tp")
                        for j in range(4):
                            nc.tensor.transpose(
                                tp[:, 128 * j:128 * (j + 1)],
                                xa[j][:, 128 * ci:128 * (ci + 1)], ident[:])
                        xt = xtq_pool.tile([128, CHUNK], BF16, tag=f"xtq{ci}",
                                           name=f"xtq{ci}")
                        nc.vector.tensor_copy(xt[:], tp[:])
                        xts.append(xt)
                    for h in range(H):
                        qp = qt_ps.tile([D, CHUNK], F32, tag="qt")
                        for ci in range(6):
                            nc.tensor.matmul(
                                qp[:], wq_t[ci][:, D * h:D * (h + 1)], xts[ci][:],
                                start=(ci == 0), stop=(ci == 5))
                        nc.scalar.copy(
                            qt_tiles[h][:, ch * CHUNK:(ch + 1) * CHUNK], qp[:])

            for ci in range(6):
                nc.sync.dma_start(wk_t[ci][:], wkt_in[128 * ci:128 * (ci + 1), :])

            # ones columns of the augmented V
            for t in range(NST):
                nc.gpsimd.memset(
                    v_tiles[t][:].rearrange("p (g c) -> p g c", c=D + 1)[:, :, D], 1.0)

            # ---- A1: per x-chunk transpose into xT + V projection (PE-dense) ----
            with tc.tile_pool(name="wv", bufs=1) as wv_pool, \
                 tc.tile_pool(name="xa", bufs=2) as xa_pool, \
                 tc.tile_pool(name="tp_ps", bufs=3, space="PSUM") as tp_ps, \
                 tc.tile_pool(name="v_ps", bufs=3, space="PSUM") as v_ps:
                kt0_state = {}
                wv_t = []
                for ci in range(6):
                    t = wv_pool.tile([128, C], BF16, tag=f"wv{ci}", name=f"wv{ci}")
                    nc.sync.dma_start(t[:], wvt_in[128 * ci:128 * (ci + 1), :])
                    wv_t.append(t)
                for ch in range(NSC):
                    xa = []
                    for j in range(4):
                        t = xa_pool.tile([128, C], BF16, tag=f"xa{j}", name=f"xa{j}")
                        nc.sync.dma_start(
                            t[:],
                            x_in[ch * CHUNK + 128 * j:ch * CHUNK + 128 * (j + 1), :])
                        xa.append(t)
                    for ci in range(6):
                        tp = tp_ps.tile([128, CHUNK], BF16, tag="tp")
                        for j in range(4):
                            nc.tensor.transpose(
                                tp[:, 128 * j:128 * (j + 1)],
                                xa[j][:, 128 * ci:128 * (ci + 1)], ident[:])
                        nc.vector.tensor_copy(
                            xT[ci][:, ch * CHUNK:(ch + 1) * CHUNK], tp[:])
                    for j in range(4):
                        for hf in range(2):
                            vp = v_ps.tile([128, 384], F32, tag="vps")
                            for ci in range(6):
                                nc.tensor.matmul(
                                    vp[:],
                                    xT[ci][:, ch * CHUNK + 128 * j:
                                           ch * CHUNK + 128 * (j + 1)],
                                    wv_t[ci][:, 384 * hf:384 * (hf + 1)],
                                    start=(ci == 0), stop=(ci == 5))
                            dst = v_tiles[4 * ch + j][:].rearrange(
                                "p (g c) -> p g c", c=D + 1)[:, 4 * hf:4 * hf + 4, 0:D]
                            nc.scalar.copy(
                                dst, vp[:].rearrange("p (g c) -> p g c", c=D))
                    project_kt_steps(0, ch, range(6), kt0_state)

            # ---- B: per head, project kT then attention (PE stays hot,
            #      ACT exp starts after the first head's kT) ----
            with tc.tile_pool(name="ex", bufs=3) as ex_pool, \
                 tc.tile_pool(name="nrm", bufs=4) as nrm_pool, \
                 tc.tile_pool(name="qk_ps", bufs=2, space="PSUM") as qk_ps, \
                 tc.tile_pool(name="pv_ps", bufs=2, space="PSUM") as pv_ps:
                NJOB = 6 * NSC     # c-steps for one head's full kT
                for h in range(H):
                    kt_state = {}
                    pv = [pv_ps.tile([D + 1, 512], F32, tag="pv", name="pv")
                          for _ in range(2)]
                    for t in range(NST):
                        # spread next head's kT projection evenly across this
                        # head's attention iterations
                        if h + 1 < H:
                            j0 = NJOB * t // NST
                            j1 = NJOB * (t + 1) // NST
                            for j in range(j0, j1):
                                sc, ci = divmod(j, 6)
                                project_kt_steps(h + 1, sc, [ci], kt_state)
                        qk = qk_ps.tile([128, 1024], F32, tag="qk")
                        for l in range(2):
                            nc.tensor.matmul(
                                qk[:, 512 * l:512 * (l + 1)],
                                kt_tiles[h][:, 128 * t:128 * (t + 1)],
                                qt_tiles[h][:, 512 * l:512 * (l + 1)],
                                start=True, stop=True)
                        ex = ex_pool.tile([128, 1024], BF16, tag="ex")
                        nc.scalar.activation(ex[:], qk[:], AF.Exp, scale=SCALE)
                        vh = v_tiles[t][:].rearrange(
                            "p (g c) -> p g c", c=D + 1)[:, h, :]
                        for l in range(2):
                            nc.tensor.matmul(
                                pv[l][:], vh, ex[:, 512 * l:512 * (l + 1)],
                                start=(t == 0), stop=(t == NST - 1))
                    for l in range(2):
                        rec = nrm_pool.tile([1, 512], F32, tag="rec", name="rec")
                        nc.vector.reciprocal(rec[:], pv[l][D:D + 1, :])
                        nc.vector.tensor_copy(
                            ao_tiles[h][:, 512 * l:512 * (l + 1)], pv[l][0:D, :])
                        bc = nrm_pool.tile([D, 512], F32, tag="bc", name="bc")
                        nc.gpsimd.partition_broadcast(bc[:], rec[:])
                        nc.vector.tensor_mul(
                            ao_tiles[h][:, 512 * l:512 * (l + 1)],
                            ao_tiles[h][:, 512 * l:512 * (l + 1)], bc[:])

        # ---- C: out-projection + LayerNorm ----
        with tc.tile_pool(name="wo", bufs=1) as wo_pool, \
             tc.tile_pool(name="ln", bufs=2) as ln_pool, \
             tc.tile_pool(name="wo_ps", bufs=3, space="PSUM") as wo_ps:
            nc.sync.dma_start(gamma_t[:], gmb_in[:])
            nc.sync.dma_start(beta_t[:], btb_in[:])
            wot_tiles = []
            for h in range(H):
                t = wo_pool.tile([D, C], BF16, tag=f"wo{h}", name=f"wo{h}")
                nc.sync.dma_start(t[:], wot_in[D * h:D * (h + 1), :])
                wot_tiles.append(t)
            for lt in range(LLOC // 128):
                wp2 = wo_ps.tile([128, C], F32, tag="wop")
                for n0, n1 in ((0, 512), (512, C)):
                    for h in range(H):
                        nc.tensor.matmul(
                            wp2[:, n0:n1],
                            ao_tiles[h][:, 128 * lt:128 * (lt + 1)],
                            wot_tiles[h][:, n0:n1],
                            start=(h == 0), stop=(h == H - 1))
                st6 = ln_pool.tile([128, 2, 6], F32, tag="st6")
                nc.vector.bn_stats(st6[:, 0, :], wp2[:, 0:384])
                nc.vector.bn_stats(st6[:, 1, :], wp2[:, 384:C])
                st2 = ln_pool.tile([128, 2], F32, tag="st2")
                nc.vector.bn_aggr(st2[:], st6[:])
                veps = ln_pool.tile([128, 1], F32, tag="veps")
                nc.vector.tensor_scalar_add(veps[:], st2[:, 1:2], EPS)
                std = ln_pool.tile([128, 1], F32, tag="std")
                nc.scalar.activation(std[:], veps[:], AF.Sqrt)
                rstd = ln_pool.tile([128, 1], F32, tag="rstd")
                nc.vector.reciprocal(rstd[:], std[:])
                negmu = ln_pool.tile([128, 1], F32, tag="negmu")
                nc.vector.tensor_scalar_mul(negmu[:], st2[:, 0:1], -1.0)
                t1 = ln_pool.tile([128, C], F32, tag="t1")
                nc.vector.scalar_tensor_tensor(
                    t1[:], wp2[:], negmu[:], gamma_t[:],
                    op0=mybir.AluOpType.add, op1=mybir.AluOpType.mult)
                ot = ln_pool.tile([128, C], F32, tag="ot")
                nc.vector.scalar_tensor_tensor(
                    ot[:], t1[:], rstd[:], beta_t[:],
                    op0=mybir.AluOpType.mult, op1=mybir.AluOpType.add)
                nc.sync.dma_start(out_ext[128 * lt:128 * (lt + 1), :], ot[:])

    nc.finalize()
    return nc


_CACHE = {}


def _get_nc():
    if "nc" not in _CACHE:
        _CACHE["nc"] = build_nc()
    return _CACHE["nc"]


def make_in_maps(h, vis, Wq, Wk, Wv, Wo, ln_gamma, ln_beta):
    import ml_dtypes
    bf16 = ml_dtypes.bfloat16
    h = np.asarray(h, np.float32).astype(bf16)
    vis = np.asarray(vis, np.float32).astype(bf16)
    wqt = np.ascontiguousarray(np.asarray(Wq, np.float32).T.astype(bf16))
    wkt = np.ascontiguousarray(np.asarray(Wk, np.float32).T.astype(bf16))
    wvt = np.ascontiguousarray(np.asarray(Wv, np.float32).T.astype(bf16))
    wot = np.ascontiguousarray(np.asarray(Wo, np.float32).T.astype(bf16))
    gmb = np.ascontiguousarray(
        np.tile(np.asarray(ln_gamma, np.float32)[None, :], (128, 1)))
    btb = np.ascontiguousarray(
        np.tile(np.asarray(ln_beta, np.float32)[None, :], (128, 1)))
    ident_np = np.eye(128, dtype=np.float32).astype(bf16)
    in_maps = []
    for core in range(N_CORES):
        b, half = core // 2, core % 2
        in_maps.append({
            "x": np.ascontiguousarray(np.concatenate([h[b], vis[b]], axis=0)),
            "hq": np.ascontiguousarray(h[b, half * LLOC:(half + 1) * LLOC]),
            "WqT": wqt, "WkT": wkt, "WvT": wvt, "WoT": wot,
            "gammab": gmb, "betab": btb, "ident": ident_np,
        })
    return in_maps


def run(in_maps, trace=False, **kw):
    nc = _get_nc()
    return run_bass_kernel_spmd(nc, in_maps, core_ids=list(range(N_CORES)),
                                trace=trace, **kw)


def assemble(results):
    full = np.empty((B, L, C), np.float32)
    for core in range(N_CORES):
        b, half = core // 2, core % 2
        full[b, half * LLOC:(half + 1) * LLOC] = results[core]["out"]
    return full


def kernel(h, vis, Wq, Wk, Wv, Wo, ln_gamma, ln_beta):
    in_maps = make_in_maps(h, vis, Wq, Wk, Wv, Wo, ln_gamma, ln_beta)
    res = run(in_maps, trace=False)
    return assemble(res.results)
